# revision 48
# baseline (speedup 1.0000x reference)
"""Causal self-attention (B=2, T=2048, C=768, H=12) on 8 TRN2 NeuronCores.

Sharding: core c -> batch b = c//4, head-group g = c%4 (heads 3g..3g+2).
Each core computes QKV for its 3 heads, causal attention, and a partial
c_proj (its heads' rows of W_proj). Host sums the 4 partials per batch.

Device layout is fully transposed (feature dim on partitions):
  xT [768, 2048], qkv^T tiles [128, 2048], scores S^T [k, q], y^T, out^T.
Softmax over k (= partition dim of S^T) uses an appended ones-column on V:
the PV matmul then yields [y_unnorm^T; denom] in one accumulation group.
No max-subtraction: scores are ~N(0,1) (|s| < ~7), exp is fp32-safe.
The denominator reciprocal uses the fast custom-DVE approximation
(~18 bits), broadcast across head-dim lanes by GpSimd.

qkv m-tile packing (host must match), 5 tiles of 128 weight cols:
  m0: [V_h0 | V_h1]   m1: [Q_h2 | V_h2]   m2: [Q_h0 | Q_h1]
  m3: [K_h0 | K_h1]   m4: [K_h2 | -]
Q_h and K_h of each head sit at the same base partition (matmul
requires equal lhsT/rhs base partitions): Q2/K2 both at base 0,
V2 rides in m1's upper half. 5 tiles instead of 6 cuts QKV matmul
cycles by 1/6 vs the naive [V|V|V|Q|Q|K|K...] padding.

identity + trimask come in as DRAM constants (DMA'd) instead of
on-device iota generation, and there is no identity-transpose warmup:
the t=0 QKV chain itself ramps the PE clock while x streams in. Input
DMAs are ordered first-needed-first: w cc0, x0 cc0, w cc1..5, x0
cc1..5, so the first QKV chain starts as soon as the sequencers reach
main (~6us) and never stalls.

Schedule is t-major with a flat, cross-head software pipeline: per
q-chunk t, QKV (5 chains) -> S-pair lookahead(2) -> V transposes
(cover the first exp latency) -> [S(i+2) | PV(i)] steady state across
all 3 heads' pair streams -> proj(t-1). S of pair i+2 always issues
before PV of pair i, so the PE never waits on the Scalar-engine exp,
including across head boundaries.

Engine placement: PE matmuls; ACT exp only; DVE trimask mults, vaug
copies, normalize; GpSimd (Pool) QKV bias-adds, proj PSUM->f16 copies,
denominator broadcast.

Output is f16, one DMA per 2 proj ct-blocks from a [128, 6ct, 512]
staging tile into a [p, t, ct, q] tiled DRAM layout (6KB contiguous
lines per partition). Host unpacks, sums partials, adds b_proj.
"""

import numpy as np
import ml_dtypes

import concourse.bass as bass
import concourse.mybir as mybir
import concourse.tile as tile
from concourse import bacc
from concourse.bass_utils import run_bass_kernel_spmd
from concourse.masks import make_identity, make_upper_triangular

F32 = mybir.dt.float32
BF16 = mybir.dt.bfloat16
F16 = mybir.dt.float16
AF = mybir.ActivationFunctionType

T = 2048           # sequence length
C = 768            # embed dim
HPC = 3            # heads per core
D = 64             # head dim
QC = 512           # q-chunk (psum bank width in fp32)
KT = 128           # k-tile
NKT = T // KT      # 16
NQC = T // QC      # 4
NCH = C // 128     # 6 contraction chunks for qkv
NMT = 5            # qkv m-tiles
WCOLS = NMT * 128  # 640 packed qkv weight cols
SCALE = 1.0 / 8.0  # 1/sqrt(64)

_CACHE = {}
LAST_RESULTS = None
_TCNT = [0]


def mk_persist(pool, shape, dtype, name=None):
    if name is None:
        _TCNT[0] += 1
        name = f"pt{_TCNT[0]}"
    return pool.tile(shape, dtype, name=name, tag=name)


def build():
    nc = bacc.Bacc("TRN2", target_bir_lowering=False)

    # w and x^T fused into one DRAM tensor: each per-cc dma_start
    # delivers the qkv weights AND the x0 chunk in one kick (2.3KB
    # lines), halving the kick count on the t0-critical path
    xwb = nc.dram_tensor("xwb", [C, WCOLS + T], BF16, kind="ExternalInput")
    battn = nc.dram_tensor("battn", [128, NMT], F32, kind="ExternalInput")
    wproj = nc.dram_tensor("wproj", [256, C], BF16, kind="ExternalInput")
    # tiled output: [p, t, ct, 512] so one DMA per ct-group writes long
    # contiguous lines per partition (DMA tail is per-line paced)
    yT = nc.dram_tensor("yT", [128, NQC, NCH, QC], F16, kind="ExternalOutput")

    with tile.TileContext(nc) as tc, \
            tc.tile_pool(name="persist", bufs=1) as pp:
        # ---- persistent SBUF tensors ----
        # identity + trimask are generated on-device (GpSimd iota is
        # ready ~7.4us -- before ANY dma data, which can't flow until
        # the ~8.6us engine-init barrier)
        ident_s = mk_persist(pp, [128, 128], F32)
        trimask_s = mk_persist(pp, [128, 128], F32)  # [k, q] = 1.0 iff k <= q
        make_identity(nc, ident_s[:, :])
        make_upper_triangular(nc, trimask_s[:, :], val=1.0, diag=True)
        ident = mk_persist(pp, [128, 128], BF16)
        trimask = mk_persist(pp, [128, 128], BF16)
        nc.vector.tensor_copy(ident[:, :], ident_s[:, :])
        nc.vector.tensor_copy(trimask[:, :], trimask_s[:, :])

        # [w (640) | x^T (2048)] per cc-chunk, one SBUF tile
        xw = mk_persist(pp, [128, NCH, WCOLS + T], BF16)
        battn_sb = mk_persist(pp, [128, NMT], F32)
        wproj_sb0 = mk_persist(pp, [128, C], BF16)   # wproj rows 0:128
        wproj_sb1 = mk_persist(pp, [128, C], BF16)   # rows 128:256 (192+ = 0)
        v01 = mk_persist(pp, [128, T], BF16)   # m0: [V_h0 | V_h1]
        q2v2 = mk_persist(pp, [128, T], BF16)  # m1: [Q_h2 | V_h2]
        qA = mk_persist(pp, [128, T], BF16)    # m2: [Q_h0 | Q_h1]
        kA = mk_persist(pp, [128, T], BF16)    # m3: [K_h0 | K_h1]
        k2 = mk_persist(pp, [128, T], BF16)    # m4: [K_h2 | -]
        vaug = mk_persist(pp, [128, NKT * HPC, 65], BF16)  # V tiles + ones col
        yA = mk_persist(pp, [128, T], BF16)    # y^T heads 0,1
        yB = mk_persist(pp, [128, T], BF16)    # y^T head 2 + zero pad

        # ---- input DMAs, first-needed-first ----
        # Each dma_start costs ~0.6us of serial kick time on its issuing
        # sequencer; sync and scalar (ACT) kick in parallel onto
        # separate ring groups. t0-critical w/x0 interleaved per-cc on
        # sync; x1 + wproj on scalar (done kicking well before the
        # first exp); x2/x3 as double-width per-cc DMAs (2KB lines).
        W0 = WCOLS  # x columns start here
        nc.sync.dma_start(battn_sb[:, :], battn[:, :])
        for cc in range(NCH):
            nc.sync.dma_start(
                xw[:, cc, 0:W0 + QC], xwb[cc * 128:(cc + 1) * 128, 0:W0 + QC])
        for cc in range(NCH):
            nc.sync.dma_start(
                xw[:, cc, W0 + QC:W0 + 2 * QC],
                xwb[cc * 128:(cc + 1) * 128, W0 + QC:W0 + 2 * QC])
        nc.sync.dma_start(wproj_sb0[:, :], wproj[0:128, :])
        nc.sync.dma_start(wproj_sb1[:, :], wproj[128:256, :])
        for cc in range(NCH):
            nc.sync.dma_start(
                xw[:, cc, W0 + 2 * QC:W0 + 4 * QC],
                xwb[cc * 128:(cc + 1) * 128, W0 + 2 * QC:W0 + 4 * QC])
        nc.vector.memset(yB[64:128, :], 0.0)

        qkv_dest = [v01, q2v2, qA, kA, k2]
        # per head: (Q tile, K tile, base row); V^T source (tile, base row)
        qk_of = [(qA, kA, 0), (qA, kA, 64), (q2v2, k2, 0)]
        vt_of = [(v01, 0), (v01, 64), (q2v2, 64)]

        with (
            tc.tile_pool(name="psS", bufs=2, space="PSUM") as psS,
            tc.tile_pool(name="psW", bufs=4, space="PSUM") as psW,
            tc.tile_pool(name="sb", bufs=4) as sbp,
            tc.tile_pool(name="sbo", bufs=2) as sbo,
        ):
            # single strided memset for every vaug ones-column
            nc.vector.memset(vaug[:, :, 64:65], 1.0)

            # a short identity-transpose burst fills the idle window
            # between PE main-dispatch and the first x/w DMA landing,
            # starting the PE clock ramp (2.4GHz after ~3us busy) early
            warm = psS.tile([128, 2 * QC], F32, tag="ps", name="ps")
            for j in range(16):
                nc.tensor.transpose(
                    warm[:, (j % 8) * 128:(j % 8) * 128 + 128],
                    ident_s[:, :], ident_s[:, :],
                )

            def emit_qkv(m, t):
                dest = qkv_dest[m]
                ps = psW.tile([128, QC], F32, tag="pw", name="pw")
                for cc in range(NCH):
                    nc.tensor.matmul(
                        ps[:, :],
                        lhsT=xw[:, cc, m * 128:(m + 1) * 128],
                        rhs=xw[:, cc, W0 + t * QC:W0 + (t + 1) * QC],
                        start=(cc == 0), stop=(cc == NCH - 1),
                    )
                # bias-add on ACT (Identity + per-partition bias AP): the
                # DVE queue stays clear of the QKV chain's PSUM-WAR path,
                # so a busy normalize chain can't stall the next chunk
                nc.scalar.activation(
                    dest[:, t * QC:(t + 1) * QC], ps[:, :],
                    AF.Identity, bias=battn_sb[:, m:m + 1],
                )

            def emit_vtrans(h, t):
                vsrc, vb = vt_of[h]
                pt = psW.tile([128, QC], F32, tag="pw", name="pw")
                ptb = pt.bitcast(BF16)
                for j in range(4):
                    kt = t * 4 + j
                    nc.tensor.transpose(
                        ptb[:, j * 64:(j + 1) * 64],
                        vsrc[vb:vb + 64, kt * KT:(kt + 1) * KT],
                        ident[vb:vb + 64, vb:vb + 64],
                    )
                vi = h * NKT + t * 4
                nc.vector.tensor_copy(
                    vaug[:, vi:vi + 4, 0:64],
                    ptb[:, 0:256].rearrange("p (a b) -> p a b", b=64),
                )

            def qlo_of(kt, t):
                dm = kt - 4 * t
                return 128 * dm if dm >= 0 else 0

            def emit_S(h, t, p):
                qt, kt_t, qb = qk_of[h]
                qlo_g = t * QC
                ps = psS.tile([128, 2 * QC], F32, tag="ps", name="ps")
                pT = sbp.tile([128, 2 * QC], BF16, tag="pT", name="pT")
                for half in range(2):
                    kt = 2 * p + half
                    qlo = qlo_of(kt, t)
                    nc.tensor.matmul(
                        ps[:, half * QC + qlo:(half + 1) * QC],
                        lhsT=kt_t[qb:qb + 64, kt * KT:(kt + 1) * KT],
                        rhs=qt[qb:qb + 64, qlo_g + qlo:qlo_g + QC],
                        start=True, stop=True,
                    )
                # one exp per k-tile pair: the ACT per-op overhead would
                # make single-tile exps the pipeline bottleneck
                lo = qlo_of(2 * p, t)
                nc.scalar.activation(
                    pT[:, lo:2 * QC], ps[:, lo:2 * QC], AF.Exp,
                    scale=SCALE,
                )
                for half in range(2):
                    kt = 2 * p + half
                    if kt - 4 * t >= 0:
                        o = half * QC + qlo_of(kt, t)
                        nc.vector.tensor_mul(
                            pT[:, o:o + 128], pT[:, o:o + 128],
                            trimask[:, :],
                        )
                return pT

            def emit_PV(h, t, p, pT, py, n_k):
                for half in range(2):
                    kt = 2 * p + half
                    qlo = qlo_of(kt, t)
                    nc.tensor.matmul(
                        py[0:65, qlo:QC],
                        lhsT=vaug[:, h * NKT + kt, :],
                        rhs=pT[:, half * QC + qlo:(half + 1) * QC],
                        start=(kt == 0), stop=(kt == n_k - 1),
                    )

            def emit_norm(h, t, py, nsplit=1):
                # den copy -> fast approx reciprocal -> GpSimd partition
                # broadcast -> multiply; nsplit>1 pipelines the chain in
                # column slices (used for the tail-critical last head)
                qlo_g = t * QC
                ydest, yrow = (yA, 0) if h == 0 else (yA, 64) if h == 1 else (yB, 0)
                w = QC // nsplit
                den = sbp.tile([1, QC], F32, tag="den", name="den")
                rec1 = sbp.tile([1, QC], F32, tag="rec1", name="rec1")
                bcast = sbp.tile([64, QC], F32, tag="bcast", name="bcast")
                # all den+recip slices first (DVE), then broadcasts
                # (GpSimd), then muls (DVE): the in-order DVE queue never
                # blocks on a GpSimd broadcast mid-chain
                for s in range(nsplit):
                    lo, hi = s * w, (s + 1) * w
                    nc.vector.tensor_copy(den[0:1, lo:hi], py[64:65, lo:hi])
                    nc.vector.reciprocal_approx_fast(
                        rec1[0:1, lo:hi], den[0:1, lo:hi])
                for s in range(nsplit):
                    lo, hi = s * w, (s + 1) * w
                    nc.gpsimd.partition_broadcast(
                        bcast[:, lo:hi], rec1[0:1, lo:hi])
                for s in range(nsplit):
                    lo, hi = s * w, (s + 1) * w
                    nc.vector.tensor_mul(
                        ydest[yrow:yrow + 64, qlo_g + lo:qlo_g + hi],
                        py[0:64, lo:hi], bcast[:, lo:hi],
                    )

            def emit_proj(ct, t, osb):
                ps = psW.tile([128, QC], F32, tag="pw", name="pw")
                nc.tensor.matmul(
                    ps[:, :],
                    lhsT=wproj_sb0[:, ct * 128:(ct + 1) * 128],
                    rhs=yA[:, t * QC:(t + 1) * QC],
                    start=True, stop=False,
                )
                nc.tensor.matmul(
                    ps[:, :],
                    lhsT=wproj_sb1[:, ct * 128:(ct + 1) * 128],
                    rhs=yB[:, t * QC:(t + 1) * QC],
                    start=False, stop=True,
                )
                nc.scalar.activation(osb[:, ct, :], ps[:, :], AF.Copy)

            def emit_proj_block(t, final=False):
                osb = sbo.tile([128, NCH, QC], F16, tag="osb", name="osb")
                for ct in range(NCH):
                    emit_proj(ct, t, osb)
                    if final and ct == NCH - 2:
                        # flush ct4 alone so it overlaps ct5's matmuls
                        nc.sync.dma_start(
                            yT[:, t, ct:ct + 1, :], osb[:, ct:ct + 1, :])
                    elif final and ct == NCH - 1:
                        # final flush: two 64-partition halves on the
                        # two kicker ring groups run in parallel,
                        # halving the last DMA's line count
                        nc.sync.dma_start(
                            yT[0:64, t, ct:ct + 1, :], osb[0:64, ct:ct + 1, :])
                        nc.scalar.dma_start(
                            yT[64:128, t, ct:ct + 1, :], osb[64:128, ct:ct + 1, :])
                    elif ct % 2 == 1:
                        # flush per 2 ct-blocks: DMA pipelines with the
                        # remaining proj matmuls, and the final
                        # completion-latency-bearing DMA is only 2KB/line
                        nc.sync.dma_start(
                            yT[:, t, ct - 1:ct + 1, :], osb[:, ct - 1:ct + 1, :])

            # ---- schedule (t-major, flat cross-head attention pipe) ----
            for t in range(NQC):
                for m in range(NMT):
                    emit_qkv(m, t)
                n_k = 4 * (t + 1)
                n_pair = n_k // 2
                # last chunk: head 2 first, so the final normalize chain
                # (which gates the tail proj block) belongs to a head
                # whose PV chain finishes early and overlaps attention
                horder = [2, 0, 1] if t == NQC - 1 else [0, 1, 2]
                work = [(h, p) for h in horder for p in range(n_pair)]
                pTs = {}
                pys = {}
                # first two S pairs up front: their exps (ACT) run ahead
                # of the proj copies, and proj + vtrans PE work covers
                # the exp latency before the first PV
                pTs[0] = emit_S(work[0][0], t, work[0][1])
                pTs[1] = emit_S(work[1][0], t, work[1][1])
                # vtrans before proj: the first PV needs the vaug copies,
                # so the transposes must not queue behind the proj block
                for h in range(HPC):
                    emit_vtrans(h, t)
                if t > 0:
                    emit_proj_block(t - 1)
                for i, (h, p) in enumerate(work):
                    if i + 2 < len(work):
                        h2_, p2_ = work[i + 2]
                        pTs[i + 2] = emit_S(h2_, t, p2_)
                    if p == 0:
                        pys[h] = psW.tile([128, QC], F32, tag="pw", name="pw")
                    emit_PV(h, t, p, pTs.pop(i), pys[h], n_k)
                    if p == n_pair - 1:
                        last = t == NQC - 1 and i == len(work) - 1
                        emit_norm(h, t, pys.pop(h), nsplit=2 if last else 1)
            emit_proj_block(NQC - 1, final=True)

    nc.finalize()
    return nc


def kernel(x, W_attn, b_attn, W_proj, b_proj):
    global LAST_RESULTS
    B = x.shape[0]
    x = np.asarray(x, np.float32)
    W_attn = np.asarray(W_attn, np.float32)
    b_attn = np.asarray(b_attn, np.float32)
    W_proj = np.asarray(W_proj, np.float32)
    b_proj = np.asarray(b_proj, np.float32)

    if "nc" not in _CACHE:
        _CACHE["nc"] = build()
    nc = _CACHE["nc"]

    in_maps = []
    for c in range(8):
        b, g = divmod(c, 4)
        heads = [3 * g + i for i in range(HPC)]
        h0, h1, h2 = heads
        Q = lambda h: W_attn[:, 64 * h:64 * h + 64]
        K = lambda h: W_attn[:, C + 64 * h:C + 64 * h + 64]
        V = lambda h: W_attn[:, 2 * C + 64 * h:2 * C + 64 * h + 64]
        bQ = lambda h: b_attn[64 * h:64 * h + 64]
        bK = lambda h: b_attn[C + 64 * h:C + 64 * h + 64]
        bV = lambda h: b_attn[2 * C + 64 * h:2 * C + 64 * h + 64]
        # m-tiles: [V0|V1], [Q2|V2], [Q0|Q1], [K0|K1], [K2|-]
        z64 = np.zeros((C, 64), np.float32)
        wqkvb = np.ascontiguousarray(np.concatenate(
            [V(h0), V(h1), Q(h2), V(h2), Q(h0), Q(h1), K(h0), K(h1),
             K(h2), z64], 1)).astype(ml_dtypes.bfloat16)
        bcols = [bV(h0), bV(h1), bQ(h2), bV(h2), bQ(h0), bQ(h1),
                 bK(h0), bK(h1), bK(h2), np.zeros(64, np.float32)]
        bvec = np.concatenate(bcols)                  # [640] = 5 x 128
        battn = np.ascontiguousarray(bvec.reshape(NMT, 128).T)  # [128, 5]
        wproj = np.zeros((256, C), np.float32)
        wproj[:192] = np.concatenate(
            [W_proj[64 * h:64 * h + 64, :] for h in heads], 0)
        xt = x[b].T.astype(ml_dtypes.bfloat16)
        xwb = np.ascontiguousarray(np.concatenate([wqkvb, xt], 1))
        in_maps.append({
            "xwb": xwb,
            "battn": battn,
            "wproj": wproj.astype(ml_dtypes.bfloat16),
        })

    res = run_bass_kernel_spmd(nc, in_maps, core_ids=list(range(8)))
    LAST_RESULTS = res

    out = np.zeros((B, T, C), np.float32)
    for c in range(8):
        b = c // 4
        # yT tiled [p, t, ct, q]: full[ct*128+p, t*512+q]
        yt = res.results[c]["yT"].astype(np.float32)
        full = yt.transpose(2, 0, 1, 3).reshape(C, T)
        out[b] += full.T
    out += b_proj
    return out


# revision 49
# speedup vs baseline: 1.0561x; 1.0561x over previous
"""Causal self-attention (B=2, T=2048, C=768, H=12) on 8 TRN2 NeuronCores.

Sharding: core c -> batch b = c//4, head-group g = c%4 (heads 3g..3g+2).
Each core computes QKV for its 3 heads, causal attention, and a partial
c_proj (its heads' rows of W_proj). Host sums the 4 partials per batch.

Device layout is fully transposed (feature dim on partitions):
  xT [768, 2048], qkv^T tiles [128, 2048], scores S^T [k, q], y^T, out^T.
Softmax over k (= partition dim of S^T) uses an appended ones-column on V:
the PV matmul then yields [y_unnorm^T; denom] in one accumulation group.
No max-subtraction: scores are ~N(0,1) (|s| < ~7), exp is fp32-safe.
The denominator reciprocal uses the fast custom-DVE approximation
(~18 bits), broadcast across head-dim lanes by GpSimd.

qkv m-tile packing (host must match), 5 tiles of 128 weight cols:
  m0: [V_h0 | V_h1]   m1: [Q_h2 | V_h2]   m2: [Q_h0 | Q_h1]
  m3: [K_h0 | K_h1]   m4: [K_h2 | -]
Q_h and K_h of each head sit at the same base partition (matmul
requires equal lhsT/rhs base partitions): Q2/K2 both at base 0,
V2 rides in m1's upper half. 5 tiles instead of 6 cuts QKV matmul
cycles by 1/6 vs the naive [V|V|V|Q|Q|K|K...] padding.

identity + trimask come in as DRAM constants (DMA'd) instead of
on-device iota generation, and there is no identity-transpose warmup:
the t=0 QKV chain itself ramps the PE clock while x streams in. Input
DMAs are ordered first-needed-first: w cc0, x0 cc0, w cc1..5, x0
cc1..5, so the first QKV chain starts as soon as the sequencers reach
main (~6us) and never stalls.

Schedule is t-major with a flat, cross-head software pipeline: per
q-chunk t, QKV (5 chains) -> S-pair lookahead(2) -> V transposes
(cover the first exp latency) -> [S(i+2) | PV(i)] steady state across
all 3 heads' pair streams -> proj(t-1). S of pair i+2 always issues
before PV of pair i, so the PE never waits on the Scalar-engine exp,
including across head boundaries.

Engine placement: PE matmuls; ACT exp only; DVE trimask mults, vaug
copies, normalize; GpSimd (Pool) QKV bias-adds, proj PSUM->f16 copies,
denominator broadcast.

Output is f16, one DMA per 2 proj ct-blocks from a [128, 6ct, 512]
staging tile into a [p, t, ct, q] tiled DRAM layout (6KB contiguous
lines per partition). Host unpacks, sums partials, adds b_proj.
"""

import numpy as np
import ml_dtypes

import concourse.bass as bass
import concourse.mybir as mybir
import concourse.tile as tile
from concourse import bacc
from concourse.bass_utils import run_bass_kernel_spmd
from concourse.masks import make_identity, make_upper_triangular

F32 = mybir.dt.float32
BF16 = mybir.dt.bfloat16
F16 = mybir.dt.float16
AF = mybir.ActivationFunctionType

T = 2048           # sequence length
C = 768            # embed dim
HPC = 3            # heads per core
D = 64             # head dim
QC = 512           # q-chunk (psum bank width in fp32)
KT = 128           # k-tile
NKT = T // KT      # 16
NQC = T // QC      # 4
NCH = C // 128     # 6 contraction chunks for qkv
NMT = 5            # qkv m-tiles
WCOLS = NMT * 128  # 640 packed qkv weight cols
SCALE = 1.0 / 8.0  # 1/sqrt(64)

_CACHE = {}
LAST_RESULTS = None
_TCNT = [0]


def mk_persist(pool, shape, dtype, name=None):
    if name is None:
        _TCNT[0] += 1
        name = f"pt{_TCNT[0]}"
    return pool.tile(shape, dtype, name=name, tag=name)


def build():
    nc = bacc.Bacc("TRN2", target_bir_lowering=False)

    # w and x^T fused into one DRAM tensor: each per-cc dma_start
    # delivers the qkv weights AND the x0 chunk in one kick (2.3KB
    # lines), halving the kick count on the t0-critical path
    xwb = nc.dram_tensor("xwb", [C, WCOLS + T], BF16, kind="ExternalInput")
    battn = nc.dram_tensor("battn", [128, NMT], F32, kind="ExternalInput")
    wproj = nc.dram_tensor("wproj", [256, C], BF16, kind="ExternalInput")
    # tiled output: [p, t, ct, 512] so one DMA per ct-group writes long
    # contiguous lines per partition (DMA tail is per-line paced)
    yT = nc.dram_tensor("yT", [128, NQC, NCH, QC], F16, kind="ExternalOutput")

    with tile.TileContext(nc) as tc, \
            tc.tile_pool(name="persist", bufs=1) as pp:
        # ---- persistent SBUF tensors ----
        # identity + trimask are generated on-device (GpSimd iota is
        # ready ~7.4us -- before ANY dma data, which can't flow until
        # the ~8.6us engine-init barrier)
        ident_s = mk_persist(pp, [128, 128], F32)
        trimask_s = mk_persist(pp, [128, 128], F32)  # [k, q] = 1.0 iff k <= q
        make_identity(nc, ident_s[:, :])
        make_upper_triangular(nc, trimask_s[:, :], val=1.0, diag=True)
        ident = mk_persist(pp, [128, 128], BF16)
        trimask = mk_persist(pp, [128, 128], BF16)
        nc.vector.tensor_copy(ident[:, :], ident_s[:, :])
        nc.vector.tensor_copy(trimask[:, :], trimask_s[:, :])

        # [w (640) | x^T (2048)] per cc-chunk, one SBUF tile
        xw = mk_persist(pp, [128, NCH, WCOLS + T], BF16)
        battn_sb = mk_persist(pp, [128, NMT], F32)
        wproj_sb0 = mk_persist(pp, [128, C], BF16)   # wproj rows 0:128
        wproj_sb1 = mk_persist(pp, [128, C], BF16)   # rows 128:256 (192+ = 0)
        v01 = mk_persist(pp, [128, T], BF16)   # m0: [V_h0 | V_h1]
        q2v2 = mk_persist(pp, [128, T], BF16)  # m1: [Q_h2 | V_h2]
        qA = mk_persist(pp, [128, T], BF16)    # m2: [Q_h0 | Q_h1]
        kA = mk_persist(pp, [128, T], BF16)    # m3: [K_h0 | K_h1]
        k2 = mk_persist(pp, [128, T], BF16)    # m4: [K_h2 | -]
        vaug = mk_persist(pp, [128, NKT * HPC, 65], BF16)  # V tiles + ones col
        yA = mk_persist(pp, [128, T], BF16)    # y^T heads 0,1
        yB = mk_persist(pp, [128, T], BF16)    # y^T head 2 + zero pad

        # ---- input DMAs, first-needed-first ----
        # Each dma_start costs ~0.6us of serial kick time on its issuing
        # sequencer; sync and scalar (ACT) kick in parallel onto
        # separate ring groups. t0-critical w/x0 interleaved per-cc on
        # sync; x1 + wproj on scalar (done kicking well before the
        # first exp); x2/x3 as double-width per-cc DMAs (2KB lines).
        W0 = WCOLS  # x columns start here
        nc.sync.dma_start(battn_sb[:, :], battn[:, :])
        for cc in range(NCH):
            nc.sync.dma_start(
                xw[:, cc, 0:W0 + QC], xwb[cc * 128:(cc + 1) * 128, 0:W0 + QC])
        for cc in range(NCH):
            nc.sync.dma_start(
                xw[:, cc, W0 + QC:W0 + 2 * QC],
                xwb[cc * 128:(cc + 1) * 128, W0 + QC:W0 + 2 * QC])
        nc.sync.dma_start(wproj_sb0[:, :], wproj[0:128, :])
        nc.sync.dma_start(wproj_sb1[:, :], wproj[128:256, :])
        for cc in range(NCH):
            nc.sync.dma_start(
                xw[:, cc, W0 + 2 * QC:W0 + 4 * QC],
                xwb[cc * 128:(cc + 1) * 128, W0 + 2 * QC:W0 + 4 * QC])
        nc.vector.memset(yB[64:128, :], 0.0)

        qkv_dest = [v01, q2v2, qA, kA, k2]
        # per head: (Q tile, K tile, base row); V^T source (tile, base row)
        qk_of = [(qA, kA, 0), (qA, kA, 64), (q2v2, k2, 0)]
        vt_of = [(v01, 0), (v01, 64), (q2v2, 64)]

        with (
            tc.tile_pool(name="psS", bufs=2, space="PSUM") as psS,
            tc.tile_pool(name="psW", bufs=4, space="PSUM") as psW,
            tc.tile_pool(name="sb", bufs=4) as sbp,
            tc.tile_pool(name="sbo", bufs=2) as sbo,
        ):
            # single strided memset for every vaug ones-column
            nc.vector.memset(vaug[:, :, 64:65], 1.0)

            # a short identity-transpose burst fills the idle window
            # between PE main-dispatch and the first x/w DMA landing,
            # starting the PE clock ramp (2.4GHz after ~3us busy) early
            warm = psS.tile([128, 2 * QC], F32, tag="ps", name="ps")
            for j in range(16):
                nc.tensor.transpose(
                    warm[:, (j % 8) * 128:(j % 8) * 128 + 128],
                    ident_s[:, :], ident_s[:, :],
                )

            def emit_qkv(m, t):
                dest = qkv_dest[m]
                ps = psW.tile([128, QC], F32, tag="pw", name="pw")
                for cc in range(NCH):
                    nc.tensor.matmul(
                        ps[:, :],
                        lhsT=xw[:, cc, m * 128:(m + 1) * 128],
                        rhs=xw[:, cc, W0 + t * QC:W0 + (t + 1) * QC],
                        start=(cc == 0), stop=(cc == NCH - 1),
                    )
                # bias-add on ACT (Identity + per-partition bias AP): the
                # DVE queue stays clear of the QKV chain's PSUM-WAR path,
                # so a busy normalize chain can't stall the next chunk
                nc.scalar.activation(
                    dest[:, t * QC:(t + 1) * QC], ps[:, :],
                    AF.Identity, bias=battn_sb[:, m:m + 1],
                )

            def emit_vtrans(h, t):
                vsrc, vb = vt_of[h]
                pt = psW.tile([128, QC], F32, tag="pw", name="pw")
                ptb = pt.bitcast(BF16)
                for j in range(4):
                    kt = t * 4 + j
                    nc.tensor.transpose(
                        ptb[:, j * 64:(j + 1) * 64],
                        vsrc[vb:vb + 64, kt * KT:(kt + 1) * KT],
                        ident[vb:vb + 64, vb:vb + 64],
                    )
                vi = h * NKT + t * 4
                nc.vector.tensor_copy(
                    vaug[:, vi:vi + 4, 0:64],
                    ptb[:, 0:256].rearrange("p (a b) -> p a b", b=64),
                )

            def qlo_of(kt, t):
                dm = kt - 4 * t
                return 128 * dm if dm >= 0 else 0

            def emit_S(h, t, p):
                qt, kt_t, qb = qk_of[h]
                qlo_g = t * QC
                ps = psS.tile([128, 2 * QC], F32, tag="ps", name="ps")
                pT = sbp.tile([128, 2 * QC], BF16, tag="pT", name="pT")
                for half in range(2):
                    kt = 2 * p + half
                    qlo = qlo_of(kt, t)
                    nc.tensor.matmul(
                        ps[:, half * QC + qlo:(half + 1) * QC],
                        lhsT=kt_t[qb:qb + 64, kt * KT:(kt + 1) * KT],
                        rhs=qt[qb:qb + 64, qlo_g + qlo:qlo_g + QC],
                        start=True, stop=True,
                    )
                # one exp per k-tile pair: the ACT per-op overhead would
                # make single-tile exps the pipeline bottleneck
                lo = qlo_of(2 * p, t)
                nc.scalar.activation(
                    pT[:, lo:2 * QC], ps[:, lo:2 * QC], AF.Exp,
                    scale=SCALE,
                )
                for half in range(2):
                    kt = 2 * p + half
                    if kt - 4 * t >= 0:
                        o = half * QC + qlo_of(kt, t)
                        nc.vector.tensor_mul(
                            pT[:, o:o + 128], pT[:, o:o + 128],
                            trimask[:, :],
                        )
                return pT

            def emit_PV(h, t, p, pT, py, n_k):
                for half in range(2):
                    kt = 2 * p + half
                    qlo = qlo_of(kt, t)
                    nc.tensor.matmul(
                        py[0:65, qlo:QC],
                        lhsT=vaug[:, h * NKT + kt, :],
                        rhs=pT[:, half * QC + qlo:(half + 1) * QC],
                        start=(kt == 0), stop=(kt == n_k - 1),
                    )

            def emit_norm(h, t, py, nsplit=1):
                # den copy -> fast approx reciprocal -> GpSimd partition
                # broadcast -> multiply; nsplit>1 pipelines the chain in
                # column slices (used for the tail-critical last head)
                qlo_g = t * QC
                ydest, yrow = (yA, 0) if h == 0 else (yA, 64) if h == 1 else (yB, 0)
                w = QC // nsplit
                den = sbp.tile([1, QC], F32, tag="den", name="den")
                rec1 = sbp.tile([1, QC], F32, tag="rec1", name="rec1")
                bcast = sbp.tile([64, QC], F32, tag="bcast", name="bcast")
                # all den+recip slices first (DVE), then broadcasts
                # (GpSimd), then muls (DVE): the in-order DVE queue never
                # blocks on a GpSimd broadcast mid-chain
                for s in range(nsplit):
                    lo, hi = s * w, (s + 1) * w
                    nc.vector.tensor_copy(den[0:1, lo:hi], py[64:65, lo:hi])
                    nc.vector.reciprocal_approx_fast(
                        rec1[0:1, lo:hi], den[0:1, lo:hi])
                for s in range(nsplit):
                    lo, hi = s * w, (s + 1) * w
                    nc.gpsimd.partition_broadcast(
                        bcast[:, lo:hi], rec1[0:1, lo:hi])
                for s in range(nsplit):
                    lo, hi = s * w, (s + 1) * w
                    nc.vector.tensor_mul(
                        ydest[yrow:yrow + 64, qlo_g + lo:qlo_g + hi],
                        py[0:64, lo:hi], bcast[:, lo:hi],
                    )

            def emit_proj(ct, t, osb):
                ps = psW.tile([128, QC], F32, tag="pw", name="pw")
                nc.tensor.matmul(
                    ps[:, :],
                    lhsT=wproj_sb0[:, ct * 128:(ct + 1) * 128],
                    rhs=yA[:, t * QC:(t + 1) * QC],
                    start=True, stop=False,
                )
                nc.tensor.matmul(
                    ps[:, :],
                    lhsT=wproj_sb1[:, ct * 128:(ct + 1) * 128],
                    rhs=yB[:, t * QC:(t + 1) * QC],
                    start=False, stop=True,
                )
                # on DVE: an ACT copy here would queue ahead of the
                # next chunk's exps and starve the attention pipeline
                nc.vector.tensor_copy(osb[:, ct, :], ps[:, :])

            def emit_proj_block(t, final=False):
                osb = sbo.tile([128, NCH, QC], F16, tag="osb", name="osb")
                for ct in range(NCH):
                    emit_proj(ct, t, osb)
                    if final and ct == NCH - 2:
                        # flush ct4 alone so it overlaps ct5's matmuls
                        nc.sync.dma_start(
                            yT[:, t, ct:ct + 1, :], osb[:, ct:ct + 1, :])
                    elif final and ct == NCH - 1:
                        # final flush: two 64-partition halves on the
                        # two kicker ring groups run in parallel,
                        # halving the last DMA's line count
                        nc.sync.dma_start(
                            yT[0:64, t, ct:ct + 1, :], osb[0:64, ct:ct + 1, :])
                        nc.scalar.dma_start(
                            yT[64:128, t, ct:ct + 1, :], osb[64:128, ct:ct + 1, :])
                    elif ct % 2 == 1:
                        # flush per 2 ct-blocks: DMA pipelines with the
                        # remaining proj matmuls, and the final
                        # completion-latency-bearing DMA is only 2KB/line
                        nc.sync.dma_start(
                            yT[:, t, ct - 1:ct + 1, :], osb[:, ct - 1:ct + 1, :])

            # ---- schedule (t-major, flat cross-head attention pipe) ----
            for t in range(NQC):
                for m in range(NMT):
                    emit_qkv(m, t)
                n_k = 4 * (t + 1)
                n_pair = n_k // 2
                # last chunk: head 2 first, so the final normalize chain
                # (which gates the tail proj block) belongs to a head
                # whose PV chain finishes early and overlaps attention
                horder = [2, 0, 1] if t == NQC - 1 else [0, 1, 2]
                work = [(h, p) for h in horder for p in range(n_pair)]
                pTs = {}
                pys = {}
                # first two S pairs up front: their exps (ACT) run ahead
                # of the proj copies, and proj + vtrans PE work covers
                # the exp latency before the first PV
                pTs[0] = emit_S(work[0][0], t, work[0][1])
                pTs[1] = emit_S(work[1][0], t, work[1][1])
                # vtrans before proj: the first PV needs the vaug copies,
                # so the transposes must not queue behind the proj block
                for h in range(HPC):
                    emit_vtrans(h, t)
                if t > 0:
                    emit_proj_block(t - 1)
                for i, (h, p) in enumerate(work):
                    if i + 2 < len(work):
                        h2_, p2_ = work[i + 2]
                        pTs[i + 2] = emit_S(h2_, t, p2_)
                    if p == 0:
                        pys[h] = psW.tile([128, QC], F32, tag="pw", name="pw")
                    emit_PV(h, t, p, pTs.pop(i), pys[h], n_k)
                    if p == n_pair - 1:
                        last = t == NQC - 1 and i == len(work) - 1
                        emit_norm(h, t, pys.pop(h), nsplit=2 if last else 1)
            emit_proj_block(NQC - 1, final=True)

    nc.finalize()
    return nc


def kernel(x, W_attn, b_attn, W_proj, b_proj):
    global LAST_RESULTS
    B = x.shape[0]
    x = np.asarray(x, np.float32)
    W_attn = np.asarray(W_attn, np.float32)
    b_attn = np.asarray(b_attn, np.float32)
    W_proj = np.asarray(W_proj, np.float32)
    b_proj = np.asarray(b_proj, np.float32)

    if "nc" not in _CACHE:
        _CACHE["nc"] = build()
    nc = _CACHE["nc"]

    in_maps = []
    for c in range(8):
        b, g = divmod(c, 4)
        heads = [3 * g + i for i in range(HPC)]
        h0, h1, h2 = heads
        Q = lambda h: W_attn[:, 64 * h:64 * h + 64]
        K = lambda h: W_attn[:, C + 64 * h:C + 64 * h + 64]
        V = lambda h: W_attn[:, 2 * C + 64 * h:2 * C + 64 * h + 64]
        bQ = lambda h: b_attn[64 * h:64 * h + 64]
        bK = lambda h: b_attn[C + 64 * h:C + 64 * h + 64]
        bV = lambda h: b_attn[2 * C + 64 * h:2 * C + 64 * h + 64]
        # m-tiles: [V0|V1], [Q2|V2], [Q0|Q1], [K0|K1], [K2|-]
        z64 = np.zeros((C, 64), np.float32)
        wqkvb = np.ascontiguousarray(np.concatenate(
            [V(h0), V(h1), Q(h2), V(h2), Q(h0), Q(h1), K(h0), K(h1),
             K(h2), z64], 1)).astype(ml_dtypes.bfloat16)
        bcols = [bV(h0), bV(h1), bQ(h2), bV(h2), bQ(h0), bQ(h1),
                 bK(h0), bK(h1), bK(h2), np.zeros(64, np.float32)]
        bvec = np.concatenate(bcols)                  # [640] = 5 x 128
        battn = np.ascontiguousarray(bvec.reshape(NMT, 128).T)  # [128, 5]
        wproj = np.zeros((256, C), np.float32)
        wproj[:192] = np.concatenate(
            [W_proj[64 * h:64 * h + 64, :] for h in heads], 0)
        xt = x[b].T.astype(ml_dtypes.bfloat16)
        xwb = np.ascontiguousarray(np.concatenate([wqkvb, xt], 1))
        in_maps.append({
            "xwb": xwb,
            "battn": battn,
            "wproj": wproj.astype(ml_dtypes.bfloat16),
        })

    res = run_bass_kernel_spmd(nc, in_maps, core_ids=list(range(8)))
    LAST_RESULTS = res

    out = np.zeros((B, T, C), np.float32)
    for c in range(8):
        b = c // 4
        # yT tiled [p, t, ct, q]: full[ct*128+p, t*512+q]
        yt = res.results[c]["yT"].astype(np.float32)
        full = yt.transpose(2, 0, 1, 3).reshape(C, T)
        out[b] += full.T
    out += b_proj
    return out
